# revision 2
# baseline (speedup 1.0000x reference)
"""Trainium2 Bass kernel for nn_CSPNet (GNN message passing), v2.

Contract: kernel(**inputs) takes FULL unsharded inputs (as in
reference.setup_inputs()) and returns the FULL [50000, 128] f32 output.

Strategy (8 NeuronCores, SPMD single program):
  - Nodes sharded into contiguous ranges of 6272 (=49 tiles of 128) per core;
    edges sharded by destination node (ei) so the scatter-mean is core-local.
  - Node features exchanged between layers as a bf16 table ([6274 rows, 128]
    per core: 6272 nodes + 2 zero rows) via AllGather into [50192, 128].
  - h[ej] (random source) is fetched with ONE 256B dma_gather descriptor per
    edge: the table is viewed as pair rows [25096, 512B]; per-core edge
    streams are parity-segregated (each tile's edges split into even-ej
    subchunks then odd-ej subchunks, uniform counts across cores), so each
    gather instruction reads the even view (stride 512B, offset 0) or the odd
    view (offset 256B) with int16 pair-row indices (< 32767).
  - h[ei] (sorted destination) is NOT gathered: edges are ei-sorted, so
    W1hi^T h[ei] is expanded on the PE: per node-tile Pt = hT_t^T @ W1hi
    [node, outf], then mm1 += Pt^T-matmul against a per-subchunk one-hot
    staircase S_T [node, edge] (cached in DRAM, built once via gpsimd
    partition_broadcast + vector EQ).
  - Edge MLP feature-major on PE (bf16, f32 PSUM); second matmul flips to
    edge-major; scatter-mean via esc-scaled staircase matmuls in PSUM.
  - Edge geometry (sinusoid embedding + lattice gram rows) built once
    on-device into a DRAM cache and streamed per layer.
  - Node MLP + residual in f32; table build (transpose+DMA) is interleaved
    into the node phase so the next layer's AllGather starts immediately and
    overlaps the per-layer Pt/staircase/geo prep.
"""

import os
import sys

sys.path.insert(0, "/opt/trn_rl_repo")

import numpy as np
import ml_dtypes

bf16 = ml_dtypes.bfloat16

import concourse.bass as bass
import concourse.bacc as bacc
import concourse.mybir as mybir
import bass_rust
from concourse import tile
from concourse.bass_utils import run_bass_kernel_spmd
from concourse.masks import make_identity

F32 = mybir.dt.float32
BF16 = mybir.dt.bfloat16
I16 = mybir.dt.int16

# ---------------- problem constants (hardcoded per contract) ----------------
N, H, B, E, L, NF = 50000, 128, 32, 800000, 4, 10
NCORES = 8
NT = 49                      # 128-node tiles per core
NPC = NT * 128               # 6272 padded nodes per core
SHARD_ROWS = NPC + 2         # + zero pair row
TROWS = SHARD_ROWS * NCORES  # 50192
PROWS = TROWS // 2           # 25096 pair rows


# ---------------- walrus workaround: <=1 sync wait per instruction ----------
def _split_excess_waits(nc, limit=1):
    work = []
    for bb in nc.main_func.blocks:
        for ins in bb.instructions:
            si = ins.sync_info
            if si is not None and si.on_wait and len(si.on_wait) > limit:
                work.append((bb, ins))
    n_added = 0
    for bb, ins in work:
        si = ins.sync_info
        w = list(si.on_wait)
        keep, extra = w[:limit], w[limit:]
        nops = []
        for i in range(0, len(extra), limit):
            nop = nc.engines[ins.engine].nop(nofuse=True)
            nop.ins.sync_info = bass_rust.SyncInfo(
                on_wait=extra[i : i + limit], on_update=[]
            )
            nops.append(nop.ins)
            n_added += 1
        si.on_wait = keep
        tail_bb = nc.cur_bb.bb if hasattr(nc.cur_bb, "bb") else nc.cur_bb
        names = {n.name for n in nops}
        tail_bb.instructions = [x for x in tail_bb.instructions if x.name not in names]
        cur = bb.instructions
        pos = next(i for i, x in enumerate(cur) if x.name == ins.name)
        bb.instructions = cur[:pos] + nops + cur[pos:]
    return n_added


# ---------------- configuration ----------------
class Cfg:
    def __init__(self, te, to, n_layers=L, group=1024, call_groups=4):
        self.te = te                      # even-parity subchunks per tile
        self.to = to                      # odd-parity subchunks per tile
        self.t_sub = te + to
        self.L = n_layers
        self.group = group
        self.gsub = group // 128
        nsub0 = NT * self.t_sub
        self.nsub = nsub0 + (-nsub0) % self.gsub
        self.ec = self.nsub * 128         # padded edges per core
        self.ng = self.nsub // self.gsub
        self.call_groups = call_groups
        self.call_e = call_groups * group
        self.call_subs = self.call_e // 128
        self.ncalls = (self.ng + call_groups - 1) // call_groups
        self.n_ngrp = (NPC + 511) // 512

    def sub_parity(self, sg):
        """Parity (0=even,1=odd) of global subchunk sg; tail pads -> 0."""
        if sg >= NT * self.t_sub:
            return 0
        return 0 if (sg % self.t_sub) < self.te else 1

    def call_runs(self, k):
        """Compile-time gather runs for call k: list of (parity, a, b) with
        a/b absolute subchunk indices, b exclusive."""
        lo = k * self.call_subs
        hi = min(lo + self.call_subs, self.nsub)
        runs = []
        for sg in range(lo, hi):
            q = self.sub_parity(sg)
            if runs and runs[-1][0] == q and runs[-1][2] == sg:
                runs[-1][2] = sg + 1
            else:
                runs.append([q, sg, sg + 1])
        return [tuple(r) for r in runs]


# ---------------- host preprocessing ----------------
def _host_prep(cfg, node_features, frac_coords, lattices, edge_index, edge2graph,
               ew1, eb1, ew2, eb2, nw1, nb1, nw2, nb2):
    ei = np.asarray(edge_index[0], np.int64)
    ej = np.asarray(edge_index[1], np.int64)
    e2g = np.asarray(edge2graph, np.int64)
    nE = ei.shape[0]
    nN = node_features.shape[0]

    frac = np.asarray(frac_coords, np.float32)
    fd_full = np.mod(frac[ej] - frac[ei], 1.0).astype(np.float32)       # [E,3]
    lat = np.asarray(lattices, np.float32)
    lat9 = np.einsum("bij,bkj->bik", lat, lat).reshape(-1, 9).astype(np.float32)
    lat_e = lat9[e2g]                                                    # [E,9]

    counts = np.bincount(ei, minlength=NCORES * NPC).astype(np.float32)
    invd = (1.0 / np.maximum(counts, 1.0)).astype(np.float32)

    gt = ei // 128                                # global tile of each edge
    qpar = (ej & 1).astype(np.int64)              # parity of source node
    key = gt * 2 + qpar
    order = np.argsort(key, kind="stable")
    eis, ejs, keys = ei[order], ej[order], key[order]
    fds, lats = fd_full[order], lat_e[order]

    ngt2 = NCORES * NT * 2
    blk_start = np.searchsorted(keys, np.arange(ngt2), side="left")
    rank = np.arange(nE) - blk_start[keys]

    te, to, t_sub = cfg.te, cfg.to, cfg.t_sub
    tloc = (gt[order] % NT)
    pos = tloc * (t_sub * 128) + qpar[order] * (te * 128) + rank
    core_of = gt[order] // NT

    ec = cfg.ec
    ZROW = PROWS - 1 + 1  # placeholder (unused)
    zrow_idx = NPC // 2   # core0's zero pair row = 3136
    per_core = []
    hT = np.zeros((NCORES, 128, NPC), np.float32)
    nf = np.asarray(node_features, np.float32)
    for c in range(NCORES):
        base = c * NPC
        hi_n = min(NPC, nN - base)
        if hi_n > 0:
            hT[c, :, :hi_n] = nf[base : base + hi_n].T

        m = core_of == c
        p = pos[m]
        loc = np.full(ec, -1.0, np.float32)       # dummy -> -1 (no scatter)
        fdv = np.zeros((ec, 3), np.float32)
        latv = np.zeros((ec, 9), np.float32)
        esc = np.zeros(ec, np.float32)
        prow = np.full(ec, zrow_idx, np.int64)    # pads -> zero pair row
        ejm = ejs[m]
        prow[p] = (ejm // NPC) * (SHARD_ROWS // 2) + (ejm % NPC) // 2
        loc[p] = (eis[m] % 128).astype(np.float32)
        esc[p] = invd[eis[m]]
        fdv[p] = fds[m]
        latv[p] = lats[m]

        ixw = prow.astype(np.int16)
        ix = np.zeros((cfg.ncalls, 128, cfg.call_e // 16), np.int16)
        for k in range(cfg.ncalls):
            seg = ixw[k * cfg.call_e : (k + 1) * cfg.call_e]
            w = np.full(cfg.call_e, zrow_idx, np.int16)
            w[: seg.shape[0]] = seg
            wt = w.reshape(cfg.call_e // 16, 16).T        # [16, ce/16]
            ix[k] = np.tile(wt, (8, 1))

        locf = np.ascontiguousarray(
            loc[: NT * t_sub * 128].reshape(NT, t_sub * 128))

        per_core.append(dict(
            hT=hT[c],
            ix_hj=ix,
            locf=locf,
            loc2=np.ascontiguousarray(loc.reshape(cfg.nsub, 128).T.astype(bf16)),
            esc=np.ascontiguousarray(esc.reshape(cfg.nsub, 128).T.astype(bf16)),
            fd_cm=np.ascontiguousarray(np.concatenate(
                [fdv, np.ones((ec, 1), np.float32)], 1)
                .reshape(cfg.ng, cfg.group, 4).transpose(0, 2, 1)),
            lat_cm=np.ascontiguousarray(
                latv.reshape(cfg.ng, cfg.group, 9).transpose(0, 2, 1).astype(bf16)),
        ))

    # shared weights
    LL = cfg.L
    ew1 = np.asarray(ew1, np.float32)
    fq2 = np.zeros((4, 60), np.float32)
    for j in range(30):
        d, f = j // NF, j % NF
        fq2[d, j] = 2.0 * np.pi * f
        fq2[d, j + 30] = 2.0 * np.pi * f
    fq2[3, :30] = np.pi                # +pi shift (mod-2pi range reduction)
    fq2[3, 30:] = np.pi + np.pi / 2    # cos rows: extra +pi/2 phase

    w1geo = np.concatenate(
        [ew1[:, 265:295], ew1[:, 295:325], ew1[:, 256:265]], axis=1)  # [L,69,128]
    nl = ew1.shape[0]
    shared = dict(
        fq2=fq2,
        w1hi=np.ascontiguousarray(ew1[:, 0:128]),
        w1hj=np.ascontiguousarray(ew1[:, 128:256]).astype(bf16),
        w1geo=np.ascontiguousarray(w1geo).astype(bf16),
        w2=np.asarray(ew2, np.float32).astype(bf16),
        nw1a=np.ascontiguousarray(np.asarray(nw1, np.float32)[:, :128]),
        nw1b=np.ascontiguousarray(np.asarray(nw1, np.float32)[:, 128:]),
        nw2=np.asarray(nw2, np.float32),
    )
    # tile weights to cfg.L layers if needed (for l8 ablation)
    if cfg.L != L:
        reps = (cfg.L + L - 1) // L
        for kk in ("w1hi", "w1hj", "w1geo", "w2", "nw1a", "nw1b", "nw2"):
            shared[kk] = np.ascontiguousarray(
                np.tile(shared[kk], (reps, 1, 1))[: cfg.L])
    in_maps = []
    for c in range(NCORES):
        m = dict(per_core[c])
        m.update(shared)
        in_maps.append(m)
    return in_maps


# ---------------- bass program ----------------
def _build(cfg, skip_gather=False, skip_compute=False, skip_ag=False,
           interleave_table=True):
    nc = bacc.Bacc("TRN2", target_bir_lowering=False)
    G = cfg.group
    GS, NG, NSUB = cfg.gsub, cfg.ng, cfg.nsub
    CE, NCALLS = cfg.call_e, cfg.ncalls
    TS = cfg.t_sub
    LL = cfg.L

    din = {}
    def inp(name, shape, dt):
        din[name] = nc.dram_tensor(name, shape, dt, kind="ExternalInput")
        return din[name]

    hT_in = inp("hT", [128, NPC], F32)
    ix_in = inp("ix_hj", [NCALLS, 128, CE // 16], I16)
    locf_in = inp("locf", [NT, TS * 128], F32)
    loc2_in = inp("loc2", [128, NSUB], BF16)
    esc_in = inp("esc", [128, NSUB], BF16)
    fd_cm = inp("fd_cm", [NG, 4, G], F32)
    lat_cm = inp("lat_cm", [NG, 9, G], BF16)
    fq2_in = inp("fq2", [4, 60], F32)
    w1hi_in = inp("w1hi", [LL, 128, 128], F32)
    w1hj_in = inp("w1hj", [LL, 128, 128], BF16)
    w1geo_in = inp("w1geo", [LL, 69, 128], BF16)
    w2_in = inp("w2", [LL, 128, 128], BF16)
    nw1a_in = inp("nw1a", [LL, 128, 128], F32)
    nw1b_in = inp("nw1b", [LL, 128, 128], F32)
    nw2_in = inp("nw2", [LL, 128, 128], F32)

    out = nc.dram_tensor("hT_out", [128, NPC], F32, kind="ExternalOutput")

    geo_cache = nc.dram_tensor("geo_cache", [NG, 69, G], BF16)
    st_cache = nc.dram_tensor("st_cache", [NT, 128, TS * 128], BF16)
    shard_dram = nc.dram_tensor("shard", [SHARD_ROWS // 2, 256], BF16)
    table = nc.dram_tensor("table", [TROWS // 2, 256], BF16, addr_space="Shared")

    Silu = mybir.ActivationFunctionType.Silu
    Sin = mybir.ActivationFunctionType.Sin
    EQ = mybir.AluOpType.is_equal

    with tile.TileContext(nc) as tc:
        with tc.tile_pool(name="persist", bufs=1) as pp:
            hT = pp.tile([128, NPC], F32)
            aggT = pp.tile([128, NPC], F32)
            zrow = pp.tile([1, 256], BF16)
            loc2 = pp.tile([128, NSUB], BF16)
            esc_t = pp.tile([128, NSUB], BF16)
            ident = pp.tile([128, 128], F32)
            iota_bf = pp.tile([128, 128], BF16)
            iota_i = pp.tile([128, 128], mybir.dt.int32)
            iota_pi = pp.tile([128, 1], mybir.dt.int32)
            iota_pf = pp.tile([128, 1], F32)
            fq2_t = pp.tile([4, 60], F32)
            negpi = pp.tile([128, 1], F32)
            twopi = pp.tile([128, 1], F32)
            w1hi_t = [pp.tile([128, 128], F32, name=f"w1hi{l}") for l in range(LL)]
            w1hj_t = [pp.tile([128, 128], BF16, name=f"w1hj{l}") for l in range(LL)]
            w1geo_t = [pp.tile([69, 128], BF16, name=f"w1geo{l}") for l in range(LL)]
            w2_t = [pp.tile([128, 128], BF16, name=f"w2{l}") for l in range(LL)]
            nw1a_t = [pp.tile([128, 128], F32, name=f"nw1a{l}") for l in range(LL)]
            nw1b_t = [pp.tile([128, 128], F32, name=f"nw1b{l}") for l in range(LL)]
            nw2_t = [pp.tile([128, 128], F32, name=f"nw2{l}") for l in range(LL)]
            pt_sb = [pp.tile([128, 128], BF16, name=f"pt{t}") for t in range(NT)]

            nc.sync.dma_start(out=hT[:], in_=hT_in[:])
            nc.gpsimd.memset(zrow[:], 0)
            nc.sync.dma_start(out=shard_dram[NPC // 2 : NPC // 2 + 1, :], in_=zrow[:])
            nc.sync.dma_start(out=loc2[:], in_=loc2_in[:])
            nc.sync.dma_start(out=esc_t[:], in_=esc_in[:])
            nc.sync.dma_start(out=fq2_t[:], in_=fq2_in[:])
            for l in range(LL):
                nc.sync.dma_start(out=w1hi_t[l][:], in_=w1hi_in[l])
                nc.sync.dma_start(out=w1hj_t[l][:], in_=w1hj_in[l])
                nc.sync.dma_start(out=w1geo_t[l][:], in_=w1geo_in[l])
                nc.sync.dma_start(out=w2_t[l][:], in_=w2_in[l])
                nc.sync.dma_start(out=nw1a_t[l][:], in_=nw1a_in[l])
                nc.sync.dma_start(out=nw1b_t[l][:], in_=nw1b_in[l])
                nc.sync.dma_start(out=nw2_t[l][:], in_=nw2_in[l])
            nc.gpsimd.memset(negpi[:], -3.14159265358979312)
            nc.gpsimd.memset(twopi[:], 6.28318530717958623)
            make_identity(nc, ident[:])
            nc.gpsimd.iota(iota_i[:], pattern=[[1, 128]], channel_multiplier=0)
            nc.vector.tensor_copy(iota_bf[:], iota_i[:])
            nc.gpsimd.iota(iota_pi[:], pattern=[[0, 1]], channel_multiplier=1)
            nc.vector.tensor_copy(iota_pf[:], iota_pi[:])

            # pair-row gather views of the table
            even_view = table[:, 0:128]
            odd_view = table[:, 128:256]

            def emit_table_tiles(tiles, tpps, tpsb):
                for t in tiles:
                    tp = tpps.tile([128, 128], F32, tag="tp")
                    nc.tensor.transpose(
                        out=tp[:], in_=hT[:, t * 128 : (t + 1) * 128],
                        identity=ident[:])
                    hnm = tpsb.tile([128, 128], BF16, tag="hnm")
                    nc.vector.tensor_copy(hnm[:], tp[:])
                    nc.sync.dma_start(
                        out=shard_dram[t * 64 : (t + 1) * 64, :], in_=hnm[:])

            def emit_ag():
                if not skip_ag:
                    nc.gpsimd.collective_compute(
                        "AllGather", mybir.AluOpType.bypass,
                        replica_groups=[list(range(NCORES))],
                        ins=[shard_dram[:]], outs=[table[:]])

            # ---- initial table (layer 0) + AG, overlapping the cache builds
            with (
                tc.tile_pool(name="tp_ps", bufs=2, space="PSUM") as tpps,
                tc.tile_pool(name="tp_sb", bufs=2) as tpsb,
            ):
                emit_table_tiles(range(NT), tpps, tpsb)
            emit_ag()

            # ---- S_T staircase cache build (once) ----
            with tc.tile_pool(name="stb", bufs=3) as stb:
                for t in range(NT):
                    lf = stb.tile([1, TS * 128], F32, tag="lf")
                    nc.sync.dma_start(out=lf[:], in_=locf_in[t : t + 1, :])
                    lb = stb.tile([128, TS * 128], F32, tag="lb")
                    nc.gpsimd.partition_broadcast(lb[:], lf[:])
                    st = stb.tile([128, TS * 128], BF16, tag="st")
                    nc.vector.tensor_tensor(
                        out=st[:], in0=lb[:],
                        in1=iota_pf[:].to_broadcast([128, TS * 128]), op=EQ)
                    nc.sync.dma_start(out=st_cache[t], in_=st[:])

            # ---- geo cache build (once) ----
            with (
                tc.tile_pool(name="gb_ps", bufs=2, space="PSUM") as gbps,
                tc.tile_pool(name="gb_sb", bufs=3) as gbsb,
            ):
                for g in range(NG):
                    fd_t = gbsb.tile([4, G], F32, tag="fd")
                    nc.sync.dma_start(out=fd_t[:], in_=fd_cm[g])
                    emb = gbps.tile([60, G], F32, tag="emb")
                    for h2 in range(G // 512):
                        sl = slice(h2 * 512, (h2 + 1) * 512)
                        nc.tensor.matmul(out=emb[:, sl], lhsT=fq2_t[:],
                                         rhs=fd_t[:, sl], start=True, stop=True)
                    uf = gbsb.tile([60, G], F32, tag="uf")
                    ki = gbsb.tile([60, G], mybir.dt.int32, tag="ki")
                    kf = gbsb.tile([60, G], F32, tag="kf")
                    nc.vector.tensor_scalar(
                        out=uf[:], in0=emb[:], scalar1=float(1 / (2 * np.pi)),
                        scalar2=None, op0=mybir.AluOpType.mult)
                    nc.vector.tensor_copy(ki[:], uf[:])
                    nc.vector.tensor_copy(kf[:], ki[:])
                    nc.vector.tensor_tensor(out=uf[:], in0=uf[:], in1=kf[:],
                                            op=mybir.AluOpType.subtract)
                    nc.vector.tensor_scalar(
                        out=kf[:], in0=uf[:], scalar1=0.0, scalar2=None,
                        op0=mybir.AluOpType.is_lt)
                    nc.vector.tensor_tensor(out=uf[:], in0=uf[:], in1=kf[:],
                                            op=mybir.AluOpType.add)
                    geo_sb = gbsb.tile([69, G], BF16, tag="geo")
                    nc.scalar.activation(geo_sb[0:60, :], uf[:], Sin,
                                         bias=negpi[0:60, :],
                                         scale=twopi[0:60, :])
                    nc.sync.dma_start(out=geo_sb[60:69, :], in_=lat_cm[g])
                    nc.sync.dma_start(out=geo_cache[g], in_=geo_sb[:])

            # ---- layers ----
            for l in range(LL):
                # Pt build: Pt[t] = hT_t^T @ W1hi  [node, outf]
                with tc.tile_pool(name=f"ptp{l}", bufs=2, space="PSUM") as ptp:
                    for t in range(NT):
                        pps = ptp.tile([128, 128], F32, tag="pt")
                        nc.tensor.matmul(
                            out=pps[:], lhsT=hT[:, t * 128 : (t + 1) * 128],
                            rhs=w1hi_t[l][:], start=True, stop=True)
                        nc.vector.tensor_copy(pt_sb[t][:], pps[:])

                # ---- edge phase ----
                with (
                    tc.tile_pool(name="mm1ps", bufs=2, space="PSUM") as mm1ps,
                    tc.tile_pool(name="mm2ps", bufs=1, space="PSUM") as mm2ps,
                    tc.tile_pool(name="aggps", bufs=2, space="PSUM") as aggps,
                    tc.tile_pool(name="gath", bufs=2) as gpool,
                    tc.tile_pool(name="stp", bufs=2) as stp,
                    tc.tile_pool(name="esb", bufs=3) as esb,
                    tc.tile_pool(name="s2p", bufs=2) as s2p,
                ):
                    agg_ps = None
                    s2_t = None
                    st_t = None
                    cur_st_tile = -1
                    for k in range(NCALLS):
                        ce = min(CE, (NG - k * cfg.call_groups) * G)
                        ixt = gpool.tile([128, ce // 16], I16, tag="ix")
                        nc.sync.dma_start(
                            out=ixt[:], in_=ix_in[k, :, : ce // 16])
                        gb = gpool.tile([128, 1, CE], BF16, tag="gb")
                        if not skip_gather:
                            for (q, a, b) in cfg.call_runs(k):
                                a0 = a - k * cfg.call_subs
                                b0 = b - k * cfg.call_subs
                                nidx = (b - a) * 128
                                nc.gpsimd.dma_gather(
                                    gb[:, :, a0 * 128 : b0 * 128],
                                    even_view if q == 0 else odd_view,
                                    ixt[:, a0 * 8 : b0 * 8], nidx, nidx,
                                    elem_size=128, elem_step=256,
                                    transpose=True, single_packet=False)
                        else:
                            nc.gpsimd.memset(gb[:, :, 0:2], 0)
                        for gg in range(ce // G):
                            g = k * cfg.call_groups + gg
                            o = gg * G
                            geo_t = esb.tile([69, G], BF16, tag="geo")
                            nc.sync.dma_start(out=geo_t[:], in_=geo_cache[g])
                            if skip_compute:
                                continue
                            mm1 = mm1ps.tile([128, G], F32, tag="mm1")
                            for h2 in range(G // 512):
                                sl = slice(h2 * 512, (h2 + 1) * 512)
                                osl = slice(o + h2 * 512, o + (h2 + 1) * 512)
                                subs = [g * GS + h2 * 4 + j for j in range(4)]
                                subs = [sg for sg in subs if sg < NT * TS]
                                nc.tensor.matmul(out=mm1[:, sl],
                                                 lhsT=w1geo_t[l][:],
                                                 rhs=geo_t[:, sl],
                                                 start=True, stop=False)
                                nc.tensor.matmul(out=mm1[:, sl],
                                                 lhsT=w1hj_t[l][:],
                                                 rhs=gb[:, 0, osl],
                                                 start=False,
                                                 stop=(not subs))
                                for idx, sg in enumerate(subs):
                                    t = sg // TS
                                    si = sg % TS
                                    if t != cur_st_tile:
                                        st_t = stp.tile([128, TS * 128], BF16,
                                                        tag="st")
                                        nc.sync.dma_start(out=st_t[:],
                                                          in_=st_cache[t])
                                        cur_st_tile = t
                                    ssl = slice((h2 * 4 + idx) * 128,
                                                (h2 * 4 + idx + 1) * 128)
                                    nc.tensor.matmul(
                                        out=mm1[:, ssl], lhsT=pt_sb[t][:],
                                        rhs=st_t[:, si * 128 : (si + 1) * 128],
                                        start=False,
                                        stop=(idx == len(subs) - 1))
                            e1 = esb.tile([128, G], BF16, tag="e1")
                            nc.scalar.activation(e1[:], mm1[:], Silu)
                            mm2 = mm2ps.tile([128, G], F32, tag="mm2")
                            for s in range(GS):
                                sl = slice(s * 128, (s + 1) * 128)
                                nc.tensor.matmul(out=mm2[:, sl],
                                                 lhsT=e1[:, sl], rhs=w2_t[l][:],
                                                 start=True, stop=True)
                            e2 = esb.tile([128, G], BF16, tag="e2")
                            nc.scalar.activation(e2[:], mm2[:], Silu)
                            for s in range(GS):
                                sg = g * GS + s
                                if sg >= NT * TS:
                                    continue
                                t = sg // TS
                                si = sg % TS
                                if si == 0:
                                    s2_t = s2p.tile([128, TS, 128], BF16,
                                                    tag="s2")
                                    nc.vector.tensor_tensor(
                                        out=s2_t[:],
                                        in0=loc2[:, t * TS : (t + 1) * TS]
                                        .unsqueeze(2)
                                        .to_broadcast([128, TS, 128]),
                                        in1=iota_bf[:, :].unsqueeze(1)
                                        .to_broadcast([128, TS, 128]),
                                        op=EQ)
                                    nc.vector.tensor_tensor(
                                        out=s2_t[:], in0=s2_t[:],
                                        in1=esc_t[:, t * TS : (t + 1) * TS]
                                        .unsqueeze(2)
                                        .to_broadcast([128, TS, 128]),
                                        op=mybir.AluOpType.mult)
                                    agg_ps = aggps.tile([128, 128], F32,
                                                        tag="agg")
                                nc.tensor.matmul(
                                    out=agg_ps[:],
                                    lhsT=e2[:, s * 128 : (s + 1) * 128],
                                    rhs=s2_t[:, si, :],
                                    start=(si == 0), stop=(si == TS - 1))
                                if si == TS - 1:
                                    nc.vector.tensor_copy(
                                        aggT[:, t * 128 : (t + 1) * 128],
                                        agg_ps[:])

                # ---- node phase (with interleaved next-layer table build) ----
                with (
                    tc.tile_pool(name="nps", bufs=2, space="PSUM") as nps,
                    tc.tile_pool(name="tp_ps", bufs=2, space="PSUM") as tpps,
                    tc.tile_pool(name="nsb", bufs=3) as nsb,
                    tc.tile_pool(name="tp_sb", bufs=2) as tpsb,
                ):
                    for g in range(cfg.n_ngrp):
                        c0 = g * 512
                        w = min(512, NPC - c0)
                        sl = slice(c0, c0 + w)
                        p1 = nps.tile([128, 512], F32, tag="np1")
                        nc.tensor.matmul(out=p1[:, :w], lhsT=nw1a_t[l][:],
                                         rhs=hT[:, sl], start=True, stop=False)
                        nc.tensor.matmul(out=p1[:, :w], lhsT=nw1b_t[l][:],
                                         rhs=hT[:, sl] if skip_compute
                                         else aggT[:, sl],
                                         start=False, stop=True)
                        o1 = nsb.tile([128, 512], F32, tag="o1")
                        nc.scalar.activation(o1[:, :w], p1[:, :w], Silu)
                        p2 = nps.tile([128, 512], F32, tag="np2")
                        nc.tensor.matmul(out=p2[:, :w], lhsT=nw2_t[l][:],
                                         rhs=o1[:, :w], start=True, stop=True)
                        o2 = nsb.tile([128, 512], F32, tag="o2")
                        nc.scalar.activation(o2[:, :w], p2[:, :w], Silu)
                        nc.vector.tensor_tensor(
                            out=hT[:, sl], in0=hT[:, sl], in1=o2[:, :w],
                            op=mybir.AluOpType.add)
                        if l < LL - 1 and interleave_table:
                            t0 = c0 // 128
                            t1 = (c0 + w + 127) // 128
                            emit_table_tiles(range(t0, min(t1, NT)), tpps, tpsb)
                    if l < LL - 1 and not interleave_table:
                        emit_table_tiles(range(NT), tpps, tpsb)
                if l < LL - 1:
                    emit_ag()

            nc.sync.dma_start(out=out[:], in_=hT[:])

    nc.compile()
    _split_excess_waits(nc, limit=1)
    bass.Bass.finalize(nc)
    return nc


# ---------------- top level ----------------
_CACHE = {}


def _get_built(cfg_key, cfg, **kw):
    if cfg_key not in _CACHE:
        _CACHE[cfg_key] = _build(cfg, **kw)
    return _CACHE[cfg_key]


def make_cfg(edge_index, n_layers=L):
    ei = np.asarray(edge_index[0], np.int64)
    ej = np.asarray(edge_index[1], np.int64)
    gt = ei // 128
    q = (ej & 1).astype(np.int64)
    cnt = np.bincount(gt * 2 + q, minlength=NCORES * NT * 2)
    te = max(1, int(np.ceil(cnt[0::2].max() / 128)))
    to = max(1, int(np.ceil(cnt[1::2].max() / 128)))
    return Cfg(te, to, n_layers=n_layers)


def kernel(**inputs):
    inputs = {k: np.asarray(v) for k, v in inputs.items()}
    cfg = make_cfg(inputs["edge_index"])
    in_maps = _host_prep(cfg, **inputs)
    nc = _get_built(("v2", cfg.te, cfg.to, cfg.L), cfg)
    res = run_bass_kernel_spmd(nc, in_maps, core_ids=list(range(NCORES)))
    outs = [res.results[c]["hT_out"] for c in range(NCORES)]
    full = np.concatenate([o.T for o in outs], axis=0)[:N]
    return np.ascontiguousarray(full.astype(np.float32))
